# revision 2
# baseline (speedup 1.0000x reference)
"""NGU episodic-novelty kNN reward kernel for 8 Trainium2 NeuronCores, v2.

Problem: for each of 64 envs, find the k=10 smallest squared distances
between obs[env] (256-d) and the first n_in_buffer[env] rows of its
8192-slot episode buffer, then compute the NGU novelty reward.

v2 strategy (memory-bound; buffer-data DMA dominates):
  - Data shipped as fp8 e4m3 (4x less than f32, 2x less than the v1
    fp16).  di becomes the exact squared distance between the QUANTIZED
    obs/buffer vectors (host subtracts |o_q|^2 back), error ~0.4% on di,
    which the batch-average normalization in the reward largely cancels.
  - Work unit = "piece": <=512 contiguous buffer slots of one env.
    Full 512-slot pieces plus one tail piece per env (rounded to 64
    slots), so shipped bytes track sum(n_in_buffer) closely.
  - Pieces sorted by length and dealt one-per-core into slots; slots are
    grouped into (Q, L) matmul groups (Q in {16,8,4,2,1} pieces, L cols)
    by a DP that minimizes shipped bytes.  The (Q, L) schedule is shared
    by all 8 cores (one SPMD program); per-core bytes are identical, so
    cores stay in lockstep.
  - Matmul: per group, Q DoubleRow fp8 matmuls. Each pump contracts all
    256 k-slots (128 partitions x 2 k-tiles) = Q blocks x 256/Q dims,
    block-diagonal weights 2*obs_q per piece. Q pumps accumulate the
    full 256-dim dot for Q pieces -> PSUM [Q, L] holds 2*o_q.b_q.
    Cost: L/2 PE cycles per piece per pump => ~107ns/512-slot piece.
    The PE output/weight width is always M=16 columns (the walrus ISA
    check rejects DoubleRow ldweights with fewer than 32 weight
    columns); groups with Q<16 zero-pad weight columns only, costing
    nothing in data DMA.
  - VectorE fuses the PSUM read with the |b_q|^2 subtract (host ships
    per-slot norm2 of the quantized data, f32); a small DMA scatters
    rows into di_sb [128, 512]; max8 + match_replace + max8 -> per-row
    top-16 candidates; DMA out cand [128, 16].
Host: per env, union of its pieces' top-16s is a superset of the true
top-k (k<=16); sort, take k, then the tiny cross-env normalization +
reward epilogue in float32.
"""

import math

import numpy as np

CAP = 8192
NENV = 64
DIM = 256
NCORES = 8
P = 128
LQ = 32                   # tail length quantum (slots)
MW = 16                   # PE output columns (DoubleRow ldweights min)
GROUP_PENALTY = 300000    # DP bytes-equivalent charge per extra group
                          # (~0.8us of per-group DGE/sem overhead at 360B/ns)
BIG_N2_F16 = 60000.0      # masked-slot norm2 that still fits in f16
NEG_BIG = -3.0e38
BIG_N2 = 1.0e9            # norm2 for masked/padded slots

EPS = 1e-3
MIN_DIST = 0.008
MAX_SIM = 2.0
L5 = 5.0

_PROGS = {}


def _f8():
    import ml_dtypes
    return ml_dtypes.float8_e4m3


def _tail_split(schedule):
    """Trailing small groups (q<=4, at most 4 of them) skip the on-device
    top-16: their raw cp rows ship to DRAM and the host selects.  Returns
    (nbig, ntail, rawq)."""
    ng = len(schedule)
    ntail = 0
    while (ntail < min(4, ng) and schedule[ng - 1 - ntail][0] <= 4):
        ntail += 1
    nbig = ng - ntail
    rawq = max((schedule[g][0] for g in range(nbig, ng)), default=0)
    return nbig, ntail, rawq


def _build_program(schedule, loop_n=None, knobs=None):
    from contextlib import ExitStack

    import concourse.bacc as bacc
    import concourse.mybir as mybir
    import concourse.tile as tile

    kn = {"bufs_loads": 0, "bufs_psums": 4, "bufs_cps": 2, "bufs_n2": 2,
          "ablate": None, "small_eng": "scalar", "load_eng": "sync",
          "n2_f16": True, "swdge_max": 1024}
    kn.update(knobs or {})

    dt = mybir.dt
    f8 = dt.float8e4

    ng = len(schedule)
    nrows = sum(q for q, _ in schedule)
    dcols = sum(2 * q * l for q, l in schedule)
    wcols = sum(2 * MW * q for q, _ in schedule)
    assert nrows <= P
    dt_n2 = dt.float16 if kn["n2_f16"] else dt.float32
    di_f16 = kn.get("di_f16", True)
    dt_di = dt.float16 if di_f16 else dt.float32
    neg_di = -float(BIG_N2_F16) if di_f16 else NEG_BIG
    nbig, ntail, rawq = _tail_split(schedule)

    # Bacc (not plain Bass): its compile() splits multi-sem waits into
    # event-semaphore instructions — the TRN2 ISA allows 1 wait per inst.
    nc = bacc.Bacc("TRN2", target_bir_lowering=False, num_devices=NCORES)
    dat = nc.dram_tensor("dat", [P, dcols], f8, kind="ExternalInput")
    wts = nc.dram_tensor("wts", [P, wcols], f8, kind="ExternalInput")
    n2t = nc.dram_tensor("n2t", [MW, ng * 512], dt_n2, kind="ExternalInput")
    cand = nc.dram_tensor("cand", [P, 16], dt_di, kind="ExternalOutput")
    if ntail:
        raw = nc.dram_tensor("raw", [MW, ntail * 512], dt_di,
                             kind="ExternalOutput")
    if kn.get("debug_out"):
        dbg_di = nc.dram_tensor("dbg_di", [P, 512], dt_di,
                                kind="ExternalOutput")
        dbg_cp = nc.dram_tensor("dbg_cp", [MW, ng * 512], dt_di,
                                kind="ExternalOutput")

    with ExitStack() as ctx:
        tc = ctx.enter_context(tile.TileContext(nc))
        consts = ctx.enter_context(tc.tile_pool(name="consts", bufs=1))
        loads = ctx.enter_context(tc.tile_pool(name="loads",
                                               bufs=max(kn["bufs_loads"], 1)))
        psums = ctx.enter_context(tc.tile_pool(name="psums",
                                               bufs=kn["bufs_psums"],
                                               space="PSUM"))
        cps = ctx.enter_context(tc.tile_pool(name="cps", bufs=kn["bufs_cps"]))
        n2s = ctx.enter_context(tc.tile_pool(name="n2s", bufs=kn["bufs_n2"]))
        outp = ctx.enter_context(tc.tile_pool(name="outp",
                                              bufs=kn.get("bufs_outp", 2)))

        small = getattr(nc, kn["small_eng"])
        load_engs = [getattr(nc, e) for e in kn["load_eng"].split(",")]
        w_sb = consts.tile([P, wcols], f8)
        small.dma_start(out=w_sb, in_=wts[:, :])

        # per-tag buffer counts: with bufs_loads=0, give every group its
        # own resident buffer (eliminates load-tile WAR stalls)
        from collections import Counter
        tag_count = Counter(q for q, _ in schedule)

        def body():
            di_sb = outp.tile([P, 512], dt_di)
            nc.gpsimd.memset(di_sb, neg_di)
            n2_sb = n2s.tile([MW, ng * 512], dt_n2, tag="n2")
            small.dma_start(out=n2_sb, in_=n2t[:, :])
            cp_b = cps.tile([MW, ng * 512], dt_di, tag="cp")
            nc.gpsimd.memset(cp_b, neg_di)
            scat = getattr(nc, kn.get("scat_eng", "scalar"))
            if ntail and kn["ablate"] is None:
                pt_tail = psums.tile([MW, ntail * 512], dt.float32,
                                     tag="ptail", bufs=1)
                nc.vector.memset(pt_tail, 0.0)

            compute = kn["ablate"] != "dmaonly"
            nfill = kn.get("fillers", 0)
            if nfill and compute:
                pt_d = psums.tile([MW, 512], dt.float32, tag="ptd", bufs=1)

            fN = min(512, wcols // 2)

            def fillers(count):
                # dummy DoubleRow matmuls on already-resident weights keep
                # the PE clock ramped while waiting for load DMAs
                for _ in range(count):
                    nc.tensor.matmul(
                        pt_d[:, 0:fN],
                        w_sb[:, 0:2 * MW].rearrange("p (i m) -> p i m", i=2),
                        w_sb[:, 0:2 * fN].rearrange("p (i j) -> p i j", i=2),
                        start=True, stop=True,
                        perf_mode=mybir.MatmulPerfMode.DoubleRow)

            doff = woff = qbase = 0

            def emit_group(gi, q, l):
                nonlocal doff, woff, qbase
                if kn["bufs_loads"] == 0:
                    t = loads.tile([P, 2, q, l], f8, tag=f"t{q}",
                                   bufs=tag_count[q] + 1, name=f"t_{gi}")
                else:
                    t = loads.tile([P, 2, q, l], f8, tag="t",
                                   bufs=kn["bufs_loads"], name=f"t_{gi}")
                if q * l <= kn["swdge_max"]:
                    le = nc.gpsimd      # SWDGE: keeps tiny loads off HWDGE
                else:
                    le = load_engs[gi % len(load_engs)]
                nsp = kn.get("load_split", 2)
                nsp = max(1, min(nsp, q))
                rstep = (q + nsp - 1) // nsp
                for r0 in range(0, q, rstep):
                    r1 = min(r0 + rstep, q)
                    le.dma_start(
                        out=t[:, :, r0:r1, :],
                        in_=dat[:, doff:doff + 2 * q * l].rearrange(
                            "p (i r j) -> p i r j", i=2, r=q)[:, :, r0:r1, :])
                if compute:
                    tail = gi >= nbig
                    if tail:
                        pt = pt_tail[:, (gi - nbig) * 512:
                                     (gi - nbig) * 512 + 512]
                    else:
                        pt = psums.tile([MW, 512], dt.float32, tag="pt",
                                        name=f"pt_{gi}")
                    for r in range(q):
                        nc.tensor.matmul(
                            pt[:, 0:l],
                            w_sb[:, woff + r * 2 * MW:
                                 woff + (r + 1) * 2 * MW].rearrange(
                                     "p (i m) -> p i m", i=2),
                            t[:, :, r, :],
                            start=(r == 0), stop=(r == q - 1),
                            perf_mode=mybir.MatmulPerfMode.DoubleRow)
                    if not tail and kn["ablate"] != "nocp":
                        # cp row = 2*o_q.b_q - |b_q|^2 = |o_q|^2 - di_q
                        # cols >= l keep the Pool memset masked
                        nc.vector.tensor_sub(
                            cp_b[0:q, gi * 512:gi * 512 + l],
                            pt[0:q, 0:l],
                            n2_sb[0:q, gi * 512:gi * 512 + l])
                        scat.dma_start(
                            out=di_sb[qbase:qbase + q, :],
                            in_=cp_b[0:q, gi * 512:(gi + 1) * 512])
                doff += 2 * q * l
                woff += 2 * MW * q
                qbase += q

            for gi in range(nbig):
                if nfill and compute:
                    fillers(kn.get("fillers0", nfill) if gi == 0 else nfill)
                emit_group(gi, *schedule[gi])

            # top-16 per big row; emitted before the tail so the DVE
            # queue is not blocked behind the tail's sub
            if kn["ablate"] is None:
                di_rep = outp.tile([P, 512], dt_di)
                cand_sb = outp.tile([P, 16], dt_di)
                nc.vector.max(out=cand_sb[:, 0:8], in_=di_sb)
                nc.vector.match_replace(out=di_rep,
                                        in_to_replace=cand_sb[:, 0:8],
                                        in_values=di_sb, imm_value=neg_di)
                nc.vector.max(out=cand_sb[:, 8:16], in_=di_rep)
                small.dma_start(out=cand[:, :], in_=cand_sb)

            for gi in range(nbig, ng):
                emit_group(gi, *schedule[gi])

            if ntail and kn["ablate"] is None:
                tc0 = nbig * 512
                nc.vector.tensor_sub(
                    cp_b[0:rawq, tc0:tc0 + ntail * 512],
                    pt_tail[0:rawq, :],
                    n2_sb[0:rawq, tc0:tc0 + ntail * 512])
                small.dma_start(out=raw[0:rawq, :],
                                in_=cp_b[0:rawq, tc0:tc0 + ntail * 512])

            if kn.get("debug_out"):
                small.dma_start(out=dbg_di[:, :], in_=di_sb)
                small.dma_start(out=dbg_cp[:, :], in_=cp_b)

        if loop_n is None:
            body()
        elif kn.get("py_unroll"):
            for _ in range(loop_n):
                body()
        else:
            with tc.For_i(0, loop_n, 1):
                body()

    nc.compile()
    return nc


def _get_program(schedule, loop_n=None, knobs=None):
    key = (tuple(schedule), loop_n,
           tuple(sorted((knobs or {}).items())))
    if key not in _PROGS:
        _PROGS[key] = _build_program(tuple(schedule), loop_n, knobs)
    return _PROGS[key]


def _ceil_lq(x):
    return max(LQ, LQ * ((x + LQ - 1) // LQ))


def _plan(n, k):
    """Build the shared (Q, L) schedule and the per-core piece table.

    Returns (assign, schedule): assign[core][slot] = (env, start, plen)
    or None; schedule = tuple of (Q, L) groups covering the slots in
    order."""
    nn = np.clip(n, 0, CAP)
    pieces = []
    for e in range(NENV):
        ne = int(nn[e])
        if ne < k:           # reference zeroes these envs
            continue
        nf, tail = divmod(ne, 512)
        for t in range(nf):
            pieces.append((e, t * 512, 512))
        if tail:
            pieces.append((e, nf * 512, tail))
    if not pieces:
        pieces.append((0, 0, 1))
    pieces.sort(key=lambda pc: -pc[2])

    ns = (len(pieces) + NCORES - 1) // NCORES   # slots per core
    assign = [[None] * ns for _ in range(NCORES)]
    lslot = [LQ] * ns
    for s in range(ns):
        grp = pieces[NCORES * s:NCORES * (s + 1)]
        lslot[s] = _ceil_lq(max(pc[2] for pc in grp))
        for j, pc in enumerate(grp):
            assign[j][s] = pc

    # DP-group the (desc-sorted) slots to minimize shipped bytes.
    # Group of q slots starting at i ships q * lslot[i] cols per block.
    sizes = (16, 8, 4, 2, 1)
    INF = float("inf")
    dp = [INF] * (ns + 1)
    choice = [0] * (ns + 1)
    dp[ns] = 0.0
    for i in range(ns - 1, -1, -1):
        for q in sizes:
            if i + q <= ns:
                c = q * lslot[i] * 256 + GROUP_PENALTY + dp[i + q]
                if c < dp[i]:
                    dp[i] = c
                    choice[i] = q
    groups = []
    s = 0
    while s < ns:
        q = choice[s]
        groups.append((q, lslot[s], list(range(s, s + q))))
        s += q
    # big groups first; taper the end so the final load->matmul->sub
    # chain after the last big DMA is short
    groups.sort(key=lambda g: -g[0])
    suffix = []
    while groups and groups[-1][0] < 8:
        suffix.insert(0, groups.pop())
    if groups and groups[-1][0] >= 8:
        q, l, sl = groups.pop()
        h, qt = q // 2, q // 4
        groups.append((h, l, sl[:h]))
        groups.append((qt, l, sl[h:h + qt]))
        groups.append((qt, l, sl[h + qt:]))
    groups.extend(suffix)
    order = [sl for g in groups for sl in g[2]]
    assign = [[row[sl] for sl in order] for row in assign]
    schedule = tuple((q, l) for q, l, _ in groups)
    return assign, schedule


def _quantize(obs, data):
    f8 = _f8()
    data_q8 = data.astype(f8)                       # [CAP, NENV, DIM]
    dq = data_q8.astype(np.float32)
    n2q = np.square(dq).sum(axis=-1)                # [CAP, NENV] f32
    obs2_q8 = (2.0 * obs).astype(f8)                # [NENV, DIM]
    oq = obs2_q8.astype(np.float32) * 0.5
    o2q = np.square(oq).sum(axis=1)                 # [NENV] |o_q|^2
    return data_q8, n2q, obs2_q8, o2q


def _make_in_maps(data_q8, n2q, obs2_q8, assign, schedule, n2_f16=True):
    f8 = _f8()
    ng = len(schedule)
    dcols = sum(2 * q * l for q, l in schedule)
    wcols = sum(2 * MW * q for q, _ in schedule)
    n2_dt = np.float16 if n2_f16 else np.float32
    n2_big = BIG_N2_F16 if n2_f16 else BIG_N2

    in_maps = []
    for m in range(NCORES):
        dat_m = np.zeros((P, dcols), f8)
        wts_m = np.zeros((P, wcols), f8)
        n2_m = np.full((MW, ng * 512), n2_big, n2_dt)
        doff = woff = qbase = 0
        for gi, (q, l) in enumerate(schedule):
            pb = P // q           # partitions per block
            dblk = np.zeros((P, 2, q, l), f8)
            wblk = np.zeros((P, q, 2, MW), f8)
            for c in range(q):
                pc = assign[m][qbase + c]
                if pc is None:
                    continue
                e, st, pl = pc
                sub = data_q8[st:st + pl, e, :]     # [pl, 256]
                # dim d = r*(256/q) + i*(128/q) + pp
                dblk[c * pb:(c + 1) * pb, :, :, :pl] = (
                    sub.reshape(pl, q, 2, pb).transpose(3, 2, 1, 0))
                wblk[c * pb:(c + 1) * pb, :, :, c] = (
                    obs2_q8[e].reshape(q, 2, pb).transpose(2, 0, 1))
                n2_m[c, gi * 512:gi * 512 + pl] = n2q[st:st + pl, e]
            dat_m[:, doff:doff + 2 * q * l] = dblk.reshape(P, 2 * q * l)
            wts_m[:, woff:woff + 2 * MW * q] = wblk.reshape(P, 2 * MW * q)
            doff += 2 * q * l
            woff += 2 * MW * q
            qbase += q
        in_maps.append({"dat": dat_m, "wts": wts_m, "n2t": n2_m})
    return in_maps


def _device_candidates(results, assign, o2q, k, schedule):
    """[NENV, k] ascending squared distances from per-core cand rows plus
    the raw-shipped tail rows.

    Device values are v = 2 o_q.b_q - |b_q|^2; di = |o_q|^2 - v."""
    nbig, ntail, rawq = _tail_split(schedule)
    rawbase = sum(q for q, _ in schedule[:nbig])
    per_env = [[] for _ in range(NENV)]
    for m in range(NCORES):
        c = np.asarray(results[m]["cand"], np.float32)      # [128, 16]
        rw = (np.asarray(results[m]["raw"], np.float32)
              if ntail else None)
        qbase = 0
        for gi, (q, l) in enumerate(schedule):
            for ci in range(q):
                pc = assign[m][qbase + ci]
                if pc is None:
                    continue
                e = pc[0]
                if gi < nbig:
                    per_env[e].append(o2q[e] - c[qbase + ci, :])
                else:
                    t0 = (gi - nbig) * 512
                    per_env[e].append(
                        o2q[e] - rw[ci, t0:t0 + min(l, pc[2])])
            qbase += q
    dists = np.zeros((NENV, k), np.float32)
    for e in range(NENV):
        if per_env[e]:
            vals = np.concatenate(per_env[e])
            vals.sort()
            dists[e] = vals[:k]
    return dists


def _epilogue(dists, r_rnd, n, k):
    f32 = np.float32
    env_valid = n >= k
    dists = np.where(env_valid[:, None], dists, f32(0.0)).astype(np.float32)
    max_d = dists[:, -1]
    cnt = env_valid.sum()
    if cnt > 0:
        avg = f32(f32((max_d * env_valid).sum(dtype=np.float32))
                  / f32(max(cnt, 1)))
    else:
        avg = f32(0.0)
    denom = avg if avg > f32(1e-5) else f32(1.0)
    dists = (dists / denom).astype(np.float32)
    dists = np.maximum(dists - f32(MIN_DIST), f32(0.0))
    kern = (f32(EPS) / (dists + f32(EPS))).astype(np.float32)
    s = np.sqrt(f32(1.0) + kern.sum(axis=1, dtype=np.float32)).astype(np.float32)
    r = np.where(s > f32(MAX_SIM), f32(0.0), f32(1.0) / s).astype(np.float32)
    modifier = np.clip(np.asarray(r_rnd, np.float32), f32(1.0), f32(L5))
    return (r * modifier).astype(np.float32)


def _run(obs, data, r_rnd, n_in_buffer, k, trace=False, knobs=None):
    from concourse.bass_utils import run_bass_kernel_spmd

    obs = np.asarray(obs, np.float32)
    data = np.asarray(data, np.float32)
    r_rnd = np.asarray(r_rnd, np.float32)
    n = np.asarray(n_in_buffer).astype(np.int64)
    k = int(k)
    assert 1 <= k <= 16, f"device top-16-per-piece only covers k<=16, got {k}"

    assign, schedule = _plan(n, k)
    nc = _get_program(schedule, knobs=knobs)
    n2_f16 = (knobs or {}).get("n2_f16", True)
    data_q8, n2q, obs2_q8, o2q = _quantize(obs, data)
    in_maps = _make_in_maps(data_q8, n2q, obs2_q8, assign, schedule,
                            n2_f16=n2_f16)
    res = run_bass_kernel_spmd(nc, in_maps, list(range(NCORES)), trace=trace)
    dists = _device_candidates(res.results, assign, o2q, k, schedule)
    return _epilogue(dists, r_rnd, n, k), res


def kernel(obs, data, r_rnd, n_in_buffer, k):
    out, _ = _run(obs, data, r_rnd, n_in_buffer, k)
    return out
